# revision 27
# baseline (speedup 1.0000x reference)
"""Trainium2 Bass kernel for nn_LogitGenerator.

Reference computation (per frame n, class c):
    proj = x @ W.T + b                       # [B*T, D]
    cos[n, c]  = <proj[n], emb[c]> / max(||proj[n]|| * ||emb[c]||, 1e-8)
    out[n, 0]  = cos[n, label[n]] / 0.1      # positive column
    out[n, 1+c] = cos[n, c] / 0.1, with -inf where emb[c] == emb[label[n]]
Rows are split into the (mask_m, mask_u) groups on the host.

Key identities: pos_cos[n] == neg_cos[n, label[n]] (the gathered positive
embedding IS emb[label[n]]); with no duplicate rows in emb the duplicate mask
is exactly (c == label[n]); the eps clamp never binds (norms ~9).

Sharding: data-parallel over batch — core i handles batch i (512 frames),
W / b / emb replicated. Host splits output rows into the two mask groups.

Toolchain constraint that shapes this kernel: every TPB instruction (and DMA)
can carry at most ONE semaphore wait. Hence:
  - operands that are consumed together ship in the same DMA (wx packs W.T
    next to x.T per k-tile; misc packs labels+iota+bias columns),
  - total DMA count stays <= 8 so no DMA needs a lane-predecessor wait,
  - tiny "absorber" ops make an engine observe a semaphore before the first
    real instruction that would otherwise need two waits.

Per-core device tensors (all fp32):
    wx   [768, 1536] 3 tiles of [256 rows]: per k-pair [W.T_k | x.T_k]
    embc [128, 1008] emb.T packed as [128, 2, 504]
    misc [128, 638]  cols 0..3 labels, 4..507 iota, 508..509 bias cols,
                     510..637 ones (row 0 used as a ones-row)
    out  [512, 505]
"""

from contextlib import ExitStack

import numpy as np

import concourse.bass as bass
import concourse.mybir as mybir
import concourse.tile as tile
from concourse.bass_utils import run_bass_kernel_spmd

B, T = 8, 512
C_ENC, D_FIN, N_CLS = 768, 256, 504
N_CORES = 8
KT = C_ENC // 128  # 6 k-tiles
JT = D_FIN // 128  # 2 d-tiles
NT = T // 128      # 4 frame-tiles per core

F32 = mybir.dt.float32
WXW = D_FIN + T        # 768 cols per k within a wx pair tile
M_LAB, M_IOTA, M_B, M_ONES = 0, NT, NT + N_CLS, NT + N_CLS + JT  # 0,4,508,510
M_COLS = M_ONES + 128  # 638

_CACHED_NC = None


def _split_multi_waits(nc: bass.Bass) -> None:
    """Walrus codegen allows ONE sem wait per instruction. Split any
    multi-wait instruction into single-wait NOPs on the same engine queue
    (FIFO order makes the conjunction equivalent) + the original with the
    last wait."""
    n = 0
    for f in nc.m.functions:
        for blk in f.blocks:
            new_insts = []
            for ins in blk.instructions:
                si = getattr(ins, "sync_info", None)
                if si is not None and si.on_wait and len(si.on_wait) > 1:
                    waits = list(si.on_wait)
                    for i, w in enumerate(waits[:-1]):
                        n += 1
                        new_insts.append(mybir.InstNoOp(
                            name=f"W-split-{n}",
                            sync_info=mybir.SyncInfo(on_wait=[w], on_update=[]),
                            bass_nofuse=True,
                            engine=ins.engine,
                        ))
                    ins.sync_info = mybir.SyncInfo(
                        on_wait=[waits[-1]], on_update=list(si.on_update))
                new_insts.append(ins)
            blk.instructions = new_insts


def build_nc(split_waits: bool = True) -> bass.Bass:
    nc = bass.Bass()

    wx = nc.declare_dram_parameter("wx", [C_ENC // 2, 2 * WXW], F32, isOutput=False)
    embc = nc.declare_dram_parameter("embc", [128, JT * N_CLS], F32, isOutput=False)
    misc = nc.declare_dram_parameter("misc", [128, M_COLS], F32, isOutput=False)
    out = nc.declare_dram_parameter("out", [T, 1 + N_CLS], F32, isOutput=True)

    with tile.TileContext(nc) as tc, ExitStack() as ctx:
        sb = ctx.enter_context(tc.tile_pool(name="sb", bufs=1))
        ps = ctx.enter_context(tc.tile_pool(name="ps", bufs=1, space="PSUM"))

        # const-AP ones: initialized at Bass construction behind an all-engine
        # barrier — zero-wait reads
        ones_col = nc.const_aps.tensor(1.0, (128, 1))
        neginf = sb.tile([128, N_CLS], F32)
        nc.vector.memset(neginf[:], float("-inf"))

        # ---- loads: 5 input DMAs total (lanes 0..4) ----
        misc_sb = sb.tile([128, M_COLS], F32)
        nc.sync.dma_start(misc_sb[:], misc[:])
        emb_sb = sb.tile([128, JT * N_CLS], F32)
        nc.sync.dma_start(emb_sb[:], embc[:])
        wx_sb = []
        for h in range(KT // 2):
            w = sb.tile([128, 2 * WXW], F32, tag=f"wx{h}")
            nc.sync.dma_start(w[:], wx[h * 128:(h + 1) * 128, :])
            wx_sb.append(w)

        def wx_lhsT(k, j):   # [128, 128] weights slice of k-tile k, d-tile j
            return wx_sb[k // 2][:, (k % 2) * WXW + j * 128:(k % 2) * WXW + (j + 1) * 128]

        def wx_rhs(k):       # [128, 512] activation slice of k-tile k
            return wx_sb[k // 2][:, (k % 2) * WXW + D_FIN:(k % 2) * WXW + WXW]

        def emb_j(j):
            return emb_sb[:, j * N_CLS:(j + 1) * N_CLS]

        # absorbers: make ACT/PE observe the misc lane via tiny reads before
        # any instruction that would otherwise need a second wait
        tmp_am = sb.tile([128, 1], F32, tag="abs_am")
        nc.scalar.copy(tmp_am[:], misc_sb[:, M_B:M_B + 1])
        dummy_ps = ps.tile([1, 1], F32)
        nc.tensor.matmul(dummy_ps[:], misc_sb[:, M_ONES:M_ONES + 1],
                         misc_sb[:, M_ONES:M_ONES + 1], start=True, stop=True)

        # one-hot masks of the label columns (f32 for reduce, u8 for predicate);
        # both operands come from the single misc DMA -> one wait each
        iota_ap = misc_sb[:, M_IOTA:M_IOTA + N_CLS]
        maskf, masku = [], []
        for t in range(NT):
            m = sb.tile([128, N_CLS], F32, tag=f"maskf{t}")
            nc.vector.tensor_scalar(m[:], iota_ap, misc_sb[:, t:t + 1], None,
                                    op0=mybir.AluOpType.is_equal)
            maskf.append(m)
            m8 = sb.tile([128, N_CLS], mybir.dt.uint8, tag=f"masku{t}")
            nc.vector.tensor_scalar(m8[:], iota_ap, misc_sb[:, t:t + 1], None,
                                    op0=mybir.AluOpType.is_equal)
            masku.append(m8)
        # DVE absorber for the emb lane (embn muls then only wait on PE)
        tmp_e = sb.tile([128, 1], F32, tag="abs_e")
        nc.vector.tensor_copy(tmp_e[:], emb_sb[:, 0:1])

        # ---- ne2[c] = sum_d emb[c, d]^2 (partition-reduce via ones-matmul) ----
        sqe = []
        for j in range(JT):
            s = sb.tile([128, N_CLS], F32, tag=f"sqe{j}")
            nc.scalar.square(s[:], emb_j(j))
            sqe.append(s)
        ne2_ps = ps.tile([1, N_CLS], F32)
        for j in range(JT):
            nc.tensor.matmul(ne2_ps[:], ones_col, sqe[j][:],
                             start=(j == 0), stop=(j == JT - 1))
        ne_row = sb.tile([1, N_CLS], F32)
        nc.scalar.sqrt(ne_row[:], ne2_ps[:])
        inv_ne = sb.tile([1, N_CLS], F32)
        nc.vector.reciprocal(inv_ne[:], ne_row[:])
        # broadcast inv_ne across partitions (K=1 matmul over the ones row)
        bcast_ps = ps.tile([128, N_CLS], F32)
        nc.tensor.matmul(bcast_ps[:], misc_sb[0:1, M_ONES:M_ONES + 128],
                         inv_ne[:], start=True, stop=True)
        embn_sb = []
        for j in range(JT):
            en = sb.tile([128, N_CLS], F32, tag=f"embn{j}")
            nc.vector.tensor_mul(en[:], emb_j(j), bcast_ps[:])
            embn_sb.append(en)

        # ---- matmul 1: projT[d, n] = b[d] + sum_k WT[k, d] * xT[k, n] ----
        projT, sqp = [], []
        for j in range(JT):
            p_ps = ps.tile([128, T], F32, tag=f"proj_ps{j}")
            for k in range(KT):
                nc.tensor.matmul(p_ps[:], wx_lhsT(k, j), wx_rhs(k),
                                 start=(k == 0), stop=(k == KT - 1))
            b_col = misc_sb[:, M_B + j:M_B + j + 1]
            pt = sb.tile([128, T], F32, tag=f"projT{j}")
            nc.vector.tensor_scalar_add(pt[:], p_ps[:], b_col)
            projT.append(pt)
            sq = sb.tile([128, T], F32, tag=f"sqp{j}")
            nc.scalar.activation(sq[:], p_ps[:],
                                 mybir.ActivationFunctionType.Square, bias=b_col)
            sqp.append(sq)

        # ---- per-frame inverse norms: inv10[n] = 10 / ||proj[n]|| ----
        nx2_ps = ps.tile([128, NT], F32)
        for t in range(NT):
            for j in range(JT):
                nc.tensor.matmul(nx2_ps[:, t:t + 1],
                                 sqp[j][:, t * 128:(t + 1) * 128], ones_col,
                                 start=(j == 0), stop=(j == JT - 1))
        s01 = sb.tile([128, NT], F32)
        nc.scalar.activation(s01[:], nx2_ps[:], mybir.ActivationFunctionType.Sqrt,
                             scale=0.01)  # sqrt(0.01*nx2) = 0.1*||proj||
        inv10 = sb.tile([128, NT], F32)
        nc.vector.reciprocal(inv10[:], s01[:])
        # ACT absorber so epilogue scale-copies wait only on PE
        tmp_a = sb.tile([128, 1], F32, tag="abs_a")
        nc.scalar.copy(tmp_a[:], inv10[:, 0:1])

        # ---- matmul 2 + epilogue, two frame-tiles per output DMA ----
        for half in range(NT // 2):
            o2 = sb.tile([128, 2, 1 + N_CLS], F32, tag=f"oh{half}")
            for tt in range(2):
                t = half * 2 + tt
                d_ps = ps.tile([128, N_CLS], F32, tag="dots_ps", bufs=2)
                for j in range(JT):
                    nc.tensor.matmul(d_ps[:], projT[j][:, t * 128:(t + 1) * 128],
                                     embn_sb[j][:], start=(j == 0),
                                     stop=(j == JT - 1))
                o = o2[:, tt, :]
                # cols 1..504: cos / 0.1  (row scale by 10/||proj[n]||)
                nc.scalar.activation(o[:, 1:], d_ps[:],
                                     mybir.ActivationFunctionType.Copy,
                                     scale=inv10[:, t:t + 1])
                # col 0: cos at the label column  (sum of cos * onehot)
                scr = sb.tile([128, N_CLS], F32, tag=f"scr{t}")
                nc.vector.tensor_mul(scr[:], o[:, 1:], maskf[t][:])
                nc.vector.reduce_sum(o[:, 0:1], scr[:], axis=mybir.AxisListType.X)
                # -inf at the label column among the negatives
                nc.vector.copy_predicated(o[:, 1:], masku[t][:], neginf[:])
            nc.sync.dma_start(
                out[half * 256:(half + 1) * 256, :].rearrange(
                    "(a p) c -> p a c", p=128),
                o2[:])

    if split_waits:  # CoreSim can't model the injected NOPs; HW needs them
        _split_multi_waits(nc)
    return nc


def _prep_inputs(x, label, W, b, label_embeddings):
    x = np.asarray(x, dtype=np.float32)
    label = np.asarray(label)
    W = np.asarray(W, dtype=np.float32)
    b = np.asarray(b, dtype=np.float32)
    emb = np.asarray(label_embeddings, dtype=np.float32)

    WT = W.T                                             # [768, 256]
    # emb.T [256, 504] packed as [128, 2*504]
    embc = np.ascontiguousarray(
        emb.T.reshape(JT, 128, N_CLS).transpose(1, 0, 2).reshape(128, JT * N_CLS))
    iota = np.broadcast_to(np.arange(N_CLS, dtype=np.float32), (128, N_CLS))
    b2 = b.reshape(JT, 128).T                            # [128, 2]
    ones = np.ones((128, 128), np.float32)

    in_maps = []
    for i in range(N_CORES):
        lab_cols = label[i].astype(np.float32).reshape(NT, 128).T  # [128, 4]
        misc = np.ascontiguousarray(
            np.concatenate([lab_cols, iota, b2, ones], axis=1))  # [128, 638]
        wxf = np.concatenate([WT, x[i].T], axis=1)       # [768, 768] = [k, WXW]
        # pair k-tiles horizontally: [3*128 rows, 2*768 cols]
        wxi = np.ascontiguousarray(
            wxf.reshape(KT // 2, 2, 128, WXW).transpose(0, 2, 1, 3)
            .reshape(KT // 2 * 128, 2 * WXW))
        in_maps.append({"wx": wxi, "embc": embc, "misc": misc})
    return in_maps


def kernel(x, label, mask_m, mask_u, W, b, label_embeddings, _trace=False):
    global _CACHED_NC
    if _CACHED_NC is None:
        _CACHED_NC = build_nc()
    nc = _CACHED_NC

    in_maps = _prep_inputs(x, label, W, b, label_embeddings)
    res = run_bass_kernel_spmd(nc, in_maps, list(range(N_CORES)), trace=_trace)

    full = np.concatenate([res.results[i]["out"][None] for i in range(N_CORES)])
    flat = full.reshape(B * T, 1 + N_CLS)
    m = np.asarray(mask_m).reshape(-1)
    u = np.asarray(mask_u).reshape(-1)
    outs = (flat[m], flat[u])
    if _trace:
        return outs, res
    return outs


# revision 28
# speedup vs baseline: 1.0695x; 1.0695x over previous
"""Trainium2 Bass kernel for nn_LogitGenerator.

Reference computation (per frame n, class c):
    proj = x @ W.T + b                       # [B*T, D]
    cos[n, c]  = <proj[n], emb[c]> / max(||proj[n]|| * ||emb[c]||, 1e-8)
    out[n, 0]  = cos[n, label[n]] / 0.1      # positive column
    out[n, 1+c] = cos[n, c] / 0.1, with -inf where emb[c] == emb[label[n]]
Rows are split into the (mask_m, mask_u) groups on the host.

Key identities: pos_cos[n] == neg_cos[n, label[n]] (the gathered positive
embedding IS emb[label[n]]); with no duplicate rows in emb the duplicate mask
is exactly (c == label[n]); the eps clamp never binds (norms ~9).

Sharding: data-parallel over batch — core i handles batch i (512 frames),
W / b / emb replicated. Host splits output rows into the two mask groups.

Toolchain note: walrus codegen accepts at most ONE semaphore wait per
instruction; _split_multi_waits() legalizes the Tile-scheduled program by
splitting multi-wait instructions into single-wait NOPs on the same queue.

Per-core device tensors (all fp32):
    wx   [768, 768]  [W.T | x.T] per k-tile row-block (one DMA per k-tile)
    embc [128, 1008] emb.T packed as [128, 2, 504]
    misc [128, 638]  cols 0..3 labels, 4..507 iota, 508..509 bias cols,
                     510..637 ones (row 0 used as a ones-row)
    out  [512, 505]
"""

from contextlib import ExitStack

import numpy as np

import concourse.bass as bass
import concourse.mybir as mybir
import concourse.tile as tile
from concourse.bass_utils import run_bass_kernel_spmd

B, T = 8, 512
C_ENC, D_FIN, N_CLS = 768, 256, 504
N_CORES = 8
KT = C_ENC // 128  # 6 k-tiles
JT = D_FIN // 128  # 2 d-tiles
NT = T // 128      # 4 frame-tiles per core

F32 = mybir.dt.float32
WXW = D_FIN + T        # 768 cols per k-tile of wx
M_LAB, M_IOTA, M_B, M_ONES = 0, NT, NT + N_CLS, NT + N_CLS + JT  # 0,4,508,510
M_COLS = M_ONES + 128  # 638

_CACHED_NC = None


def _split_multi_waits(nc: bass.Bass) -> None:
    """Walrus codegen allows ONE sem wait per instruction. Split any
    multi-wait instruction into single-wait NOPs on the same engine queue
    (FIFO order makes the conjunction equivalent) + the original with the
    last wait."""
    n = 0
    for f in nc.m.functions:
        for blk in f.blocks:
            new_insts = []
            for ins in blk.instructions:
                si = getattr(ins, "sync_info", None)
                if si is not None and si.on_wait and len(si.on_wait) > 1:
                    waits = list(si.on_wait)
                    for i, w in enumerate(waits[:-1]):
                        n += 1
                        new_insts.append(mybir.InstNoOp(
                            name=f"W-split-{n}",
                            sync_info=mybir.SyncInfo(on_wait=[w], on_update=[]),
                            bass_nofuse=True,
                            engine=ins.engine,
                        ))
                    ins.sync_info = mybir.SyncInfo(
                        on_wait=[waits[-1]], on_update=list(si.on_update))
                new_insts.append(ins)
            blk.instructions = new_insts


def build_nc(split_waits: bool = True) -> bass.Bass:
    nc = bass.Bass()

    wx = nc.declare_dram_parameter("wx", [C_ENC, WXW], F32, isOutput=False)
    embc = nc.declare_dram_parameter("embc", [128, JT * N_CLS], F32, isOutput=False)
    misc = nc.declare_dram_parameter("misc", [128, M_COLS], F32, isOutput=False)
    out = nc.declare_dram_parameter("out", [T, 1 + N_CLS], F32, isOutput=True)

    with tile.TileContext(nc) as tc, ExitStack() as ctx:
        sb = ctx.enter_context(tc.tile_pool(name="sb", bufs=1))
        ps = ctx.enter_context(tc.tile_pool(name="ps", bufs=1, space="PSUM"))

        ones_col = nc.const_aps.tensor(1.0, (128, 1))
        neginf = sb.tile([128, N_CLS], F32)
        nc.vector.memset(neginf[:], float("-inf"))

        # ---- loads: small tensors first, then per-k wx tiles ----
        emb_sb = sb.tile([128, JT * N_CLS], F32)
        nc.sync.dma_start(emb_sb[:], embc[:])
        misc_sb = sb.tile([128, M_COLS], F32)
        nc.sync.dma_start(misc_sb[:], misc[:])
        wx_sb = []
        for k in range(KT):
            w = sb.tile([128, WXW], F32, tag=f"wx{k}")
            nc.sync.dma_start(w[:], wx[k * 128:(k + 1) * 128, :])
            wx_sb.append(w)

        def emb_j(j):
            return emb_sb[:, j * N_CLS:(j + 1) * N_CLS]

        # one-hot masks of the label columns (u8; DVE converts on read)
        iota_ap = misc_sb[:, M_IOTA:M_IOTA + N_CLS]
        masku = []
        for t in range(NT):
            m8 = sb.tile([128, N_CLS], mybir.dt.uint8, tag=f"masku{t}")
            nc.vector.tensor_scalar(m8[:], iota_ap, misc_sb[:, t:t + 1], None,
                                    op0=mybir.AluOpType.is_equal)
            masku.append(m8)

        # ---- ne2[c] = sum_d emb[c, d]^2 (partition-reduce via ones-matmul) ----
        sqe = []
        for j in range(JT):
            s = sb.tile([128, N_CLS], F32, tag=f"sqe{j}")
            nc.scalar.square(s[:], emb_j(j))
            sqe.append(s)
        ne2_ps = ps.tile([1, N_CLS], F32)
        for j in range(JT):
            nc.tensor.matmul(ne2_ps[:], ones_col, sqe[j][:],
                             start=(j == 0), stop=(j == JT - 1))
        ne_row = sb.tile([1, N_CLS], F32)
        nc.scalar.sqrt(ne_row[:], ne2_ps[:])
        inv_ne = sb.tile([1, N_CLS], F32)
        nc.vector.reciprocal(inv_ne[:], ne_row[:])
        # broadcast inv_ne across partitions (K=1 matmul over the ones row)
        bcast_ps = ps.tile([128, N_CLS], F32)
        nc.tensor.matmul(bcast_ps[:], misc_sb[0:1, M_ONES:M_ONES + 128],
                         inv_ne[:], start=True, stop=True)
        embn_sb = []
        for j in range(JT):
            en = sb.tile([128, N_CLS], F32, tag=f"embn{j}")
            nc.vector.tensor_mul(en[:], emb_j(j), bcast_ps[:])
            embn_sb.append(en)

        # ---- matmul 1: projT[d, n] = b[d] + sum_k WT[k, d] * xT[k, n] ----
        projT, sqp = [], []
        for j in range(JT):
            p_ps = ps.tile([128, T], F32, tag=f"proj_ps{j}")
            for k in range(KT):
                nc.tensor.matmul(p_ps[:],
                                 wx_sb[k][:, j * 128:(j + 1) * 128],
                                 wx_sb[k][:, D_FIN:],
                                 start=(k == 0), stop=(k == KT - 1))
            b_col = misc_sb[:, M_B + j:M_B + j + 1]
            pt = sb.tile([128, T], F32, tag=f"projT{j}")
            nc.vector.tensor_scalar_add(pt[:], p_ps[:], b_col)
            projT.append(pt)
            sq = sb.tile([128, T], F32, tag=f"sqp{j}")
            nc.scalar.activation(sq[:], p_ps[:],
                                 mybir.ActivationFunctionType.Square, bias=b_col)
            sqp.append(sq)

        # ---- per-frame inverse norms: inv10[n] = 10 / ||proj[n]|| ----
        nx2_ps = ps.tile([128, NT], F32)
        for t in range(NT):
            for j in range(JT):
                nc.tensor.matmul(nx2_ps[:, t:t + 1],
                                 sqp[j][:, t * 128:(t + 1) * 128], ones_col,
                                 start=(j == 0), stop=(j == JT - 1))
        s01 = sb.tile([128, NT], F32)
        nc.scalar.activation(s01[:], nx2_ps[:], mybir.ActivationFunctionType.Sqrt,
                             scale=0.01)  # sqrt(0.01*nx2) = 0.1*||proj||
        inv10 = sb.tile([128, NT], F32)
        nc.vector.reciprocal(inv10[:], s01[:])

        # ---- matmul 2 + epilogue per frame-tile ----
        for t in range(NT):
            d_ps = ps.tile([128, N_CLS], F32, tag="dots_ps", bufs=2)
            for j in range(JT):
                nc.tensor.matmul(d_ps[:], projT[j][:, t * 128:(t + 1) * 128],
                                 embn_sb[j][:], start=(j == 0),
                                 stop=(j == JT - 1))
            o = sb.tile([128, 1 + N_CLS], F32, tag=f"o{t}")
            # cols 1..504: cos / 0.1  (row scale by 10/||proj[n]||)
            nc.scalar.activation(o[:, 1:], d_ps[:],
                                 mybir.ActivationFunctionType.Copy,
                                 scale=inv10[:, t:t + 1])
            # col 0: cos at the label column  (sum of cos * onehot)
            scr = sb.tile([128, N_CLS], F32, tag=f"scr{t}")
            nc.vector.tensor_mul(scr[:], o[:, 1:], masku[t][:])
            nc.vector.reduce_sum(o[:, 0:1], scr[:], axis=mybir.AxisListType.X)
            # -inf at the label column among the negatives
            nc.vector.copy_predicated(o[:, 1:], masku[t][:], neginf[:])
            nc.sync.dma_start(out[t * 128:(t + 1) * 128, :], o[:])

    if split_waits:  # CoreSim can't model the injected NOPs; HW needs them
        _split_multi_waits(nc)
    return nc


def _prep_inputs(x, label, W, b, label_embeddings):
    x = np.asarray(x, dtype=np.float32)
    label = np.asarray(label)
    W = np.asarray(W, dtype=np.float32)
    b = np.asarray(b, dtype=np.float32)
    emb = np.asarray(label_embeddings, dtype=np.float32)

    WT = W.T                                             # [768, 256]
    embc = np.ascontiguousarray(
        emb.T.reshape(JT, 128, N_CLS).transpose(1, 0, 2).reshape(128, JT * N_CLS))
    iota = np.broadcast_to(np.arange(N_CLS, dtype=np.float32), (128, N_CLS))
    b2 = b.reshape(JT, 128).T                            # [128, 2]
    ones = np.ones((128, 128), np.float32)

    in_maps = []
    for i in range(N_CORES):
        lab_cols = label[i].astype(np.float32).reshape(NT, 128).T  # [128, 4]
        misc = np.ascontiguousarray(
            np.concatenate([lab_cols, iota, b2, ones], axis=1))  # [128, 638]
        wxi = np.ascontiguousarray(np.concatenate([WT, x[i].T], axis=1))
        in_maps.append({"wx": wxi, "embc": embc, "misc": misc})
    return in_maps


def kernel(x, label, mask_m, mask_u, W, b, label_embeddings, _trace=False):
    global _CACHED_NC
    if _CACHED_NC is None:
        _CACHED_NC = build_nc()
    nc = _CACHED_NC

    in_maps = _prep_inputs(x, label, W, b, label_embeddings)
    res = run_bass_kernel_spmd(nc, in_maps, list(range(N_CORES)), trace=_trace)

    full = np.concatenate([res.results[i]["out"][None] for i in range(N_CORES)])
    flat = full.reshape(B * T, 1 + N_CLS)
    m = np.asarray(mask_m).reshape(-1)
    u = np.asarray(mask_u).reshape(-1)
    outs = (flat[m], flat[u])
    if _trace:
        return outs, res
    return outs


# revision 32
# speedup vs baseline: 1.0914x; 1.0205x over previous
"""Trainium2 Bass kernel for nn_LogitGenerator.

Reference computation (per frame n, class c):
    proj = x @ W.T + b                       # [B*T, D]
    cos[n, c]  = <proj[n], emb[c]> / max(||proj[n]|| * ||emb[c]||, 1e-8)
    out[n, 0]  = cos[n, label[n]] / 0.1      # positive column
    out[n, 1+c] = cos[n, c] / 0.1, with -inf where emb[c] == emb[label[n]]
Rows are split into the (mask_m, mask_u) groups on the host.

Key identities: pos_cos[n] == neg_cos[n, label[n]] (the gathered positive
embedding IS emb[label[n]]); with no duplicate rows in emb the duplicate mask
is exactly (c == label[n]); the eps clamp never binds (norms ~9).

Sharding: data-parallel over batch — core i handles batch i (512 frames),
W / b / emb replicated. Host splits output rows into the two mask groups.

Toolchain note: walrus codegen accepts at most ONE semaphore wait per
instruction; _split_multi_waits() legalizes the Tile-scheduled program by
splitting multi-wait instructions into single-wait NOPs on the same queue.

Per-core device tensors (all fp32):
    wx   [768, 768]  [W.T | x.T] per k-tile row-block (one DMA per k-tile)
    embc [128, 1008] emb.T packed as [128, 2, 504]
    misc [128, 638]  cols 0..3 labels, 4..507 iota, 508..509 bias cols,
                     510..637 ones (row 0 used as a ones-row)
    out  [512, 505]
"""

from contextlib import ExitStack

import numpy as np

import concourse.bass as bass
import concourse.mybir as mybir
import concourse.tile as tile
from concourse.bass_utils import run_bass_kernel_spmd

B, T = 8, 512
C_ENC, D_FIN, N_CLS = 768, 256, 504
N_CORES = 8
KT = C_ENC // 128  # 6 k-tiles
JT = D_FIN // 128  # 2 d-tiles
NT = T // 128      # 4 frame-tiles per core

F32 = mybir.dt.float32
WXW = D_FIN + T        # 768 cols per k-tile of wx
M_LAB, M_IOTA, M_B, M_ONES = 0, NT, NT + N_CLS, NT + N_CLS + JT  # 0,4,508,510
M_COLS = M_ONES + 128  # 638

_CACHED_NC = None


def _split_multi_waits(nc: bass.Bass) -> None:
    """Walrus codegen allows ONE sem wait per instruction. Split any
    multi-wait instruction into single-wait NOPs on the same engine queue
    (FIFO order makes the conjunction equivalent) + the original with the
    last wait."""
    n = 0
    for f in nc.m.functions:
        for blk in f.blocks:
            new_insts = []
            for ins in blk.instructions:
                si = getattr(ins, "sync_info", None)
                if si is not None and si.on_wait and len(si.on_wait) > 1:
                    waits = list(si.on_wait)
                    for i, w in enumerate(waits[:-1]):
                        n += 1
                        new_insts.append(mybir.InstNoOp(
                            name=f"W-split-{n}",
                            sync_info=mybir.SyncInfo(on_wait=[w], on_update=[]),
                            bass_nofuse=True,
                            engine=ins.engine,
                        ))
                    ins.sync_info = mybir.SyncInfo(
                        on_wait=[waits[-1]], on_update=list(si.on_update))
                new_insts.append(ins)
            blk.instructions = new_insts


def build_nc(split_waits: bool = True) -> bass.Bass:
    nc = bass.Bass()

    wx = nc.declare_dram_parameter("wx", [C_ENC, WXW], F32, isOutput=False)
    embc = nc.declare_dram_parameter("embc", [128, JT * N_CLS], F32, isOutput=False)
    misc = nc.declare_dram_parameter("misc", [128, M_COLS], F32, isOutput=False)
    out = nc.declare_dram_parameter("out", [T, 1 + N_CLS], F32, isOutput=True)

    with tile.TileContext(nc) as tc, ExitStack() as ctx:
        sb = ctx.enter_context(tc.tile_pool(name="sb", bufs=1))
        ps = ctx.enter_context(tc.tile_pool(name="ps", bufs=1, space="PSUM"))

        ones_col = nc.const_aps.tensor(1.0, (128, 1))
        neginf = sb.tile([128, N_CLS], F32)
        nc.vector.memset(neginf[:], float("-inf"))

        # ---- loads ----
        # wx tiles stream on the SP HWDGE queue (FIFO: k-tiles arrive in
        # order, matmul1 starts as soon as wx0 lands); the small tensors go
        # on the ACT HWDGE queue so their issue/transfer overlaps wx.
        wx_sb = []
        for k in range(KT):
            w = sb.tile([128, WXW], F32, tag=f"wx{k}")
            nc.sync.dma_start(w[:], wx[k * 128:(k + 1) * 128, :])
            wx_sb.append(w)
        emb_sb = sb.tile([128, JT * N_CLS], F32)
        nc.scalar.dma_start(emb_sb[:], embc[:])
        misc_sb = sb.tile([128, M_COLS], F32)
        nc.scalar.dma_start(misc_sb[:], misc[:])

        def emb_j(j):
            return emb_sb[:, j * N_CLS:(j + 1) * N_CLS]

        # one-hot masks of the label columns (u8; DVE converts on read)
        iota_ap = misc_sb[:, M_IOTA:M_IOTA + N_CLS]
        masku = []
        for t in range(NT):
            m8 = sb.tile([128, N_CLS], mybir.dt.uint8, tag=f"masku{t}")
            nc.vector.tensor_scalar(m8[:], iota_ap, misc_sb[:, t:t + 1], None,
                                    op0=mybir.AluOpType.is_equal)
            masku.append(m8)

        # ---- ne2[c] = sum_d emb[c, d]^2 (partition-reduce via ones-matmul) ----
        sqe = []
        for j in range(JT):
            s = sb.tile([128, N_CLS], F32, tag=f"sqe{j}")
            nc.scalar.square(s[:], emb_j(j))
            sqe.append(s)
        ne2_ps = ps.tile([1, N_CLS], F32)
        for j in range(JT):
            nc.tensor.matmul(ne2_ps[:], ones_col, sqe[j][:],
                             start=(j == 0), stop=(j == JT - 1))
        ne_row = sb.tile([1, N_CLS], F32)
        nc.scalar.sqrt(ne_row[:], ne2_ps[:])
        inv_ne = sb.tile([1, N_CLS], F32)
        nc.vector.reciprocal(inv_ne[:], ne_row[:])
        # broadcast inv_ne across partitions (K=1 matmul over the ones row)
        bcast_ps = ps.tile([128, N_CLS], F32)
        nc.tensor.matmul(bcast_ps[:], misc_sb[0:1, M_ONES:M_ONES + 128],
                         inv_ne[:], start=True, stop=True)
        embn_sb = []
        for j in range(JT):
            en = sb.tile([128, N_CLS], F32, tag=f"embn{j}")
            nc.vector.tensor_mul(en[:], emb_j(j), bcast_ps[:])
            embn_sb.append(en)

        # ---- matmul 1: projT[d, n] = b[d] + sum_k WT[k, d] * xT[k, n] ----
        # k-outer / j-inner: each arriving wx tile is consumed by two
        # back-to-back matmuls, matching DMA delivery rate (keeps PE warm)
        p_ps = [ps.tile([128, T], F32, tag=f"proj_ps{j}", name=f"proj_ps{j}")
                for j in range(JT)]
        for k in range(KT):
            for j in range(JT):
                nc.tensor.matmul(p_ps[j][:],
                                 wx_sb[k][:, j * 128:(j + 1) * 128],
                                 wx_sb[k][:, D_FIN:],
                                 start=(k == 0), stop=(k == KT - 1))
        projT, sqp = [], []
        for j in range(JT):
            b_col = misc_sb[:, M_B + j:M_B + j + 1]
            pt = sb.tile([128, T], F32, tag=f"projT{j}")
            nc.vector.tensor_scalar_add(pt[:], p_ps[j][:], b_col)
            projT.append(pt)
            sq = sb.tile([128, T], F32, tag=f"sqp{j}")
            nc.scalar.activation(sq[:], p_ps[j][:],
                                 mybir.ActivationFunctionType.Square, bias=b_col)
            sqp.append(sq)

        # ---- per-frame inverse norms: inv10[n] = 10 / ||proj[n]|| ----
        nx2_ps = ps.tile([128, NT], F32)
        for t in range(NT):
            for j in range(JT):
                nc.tensor.matmul(nx2_ps[:, t:t + 1],
                                 sqp[j][:, t * 128:(t + 1) * 128], ones_col,
                                 start=(j == 0), stop=(j == JT - 1))
        s01 = sb.tile([128, NT], F32)
        nc.scalar.activation(s01[:], nx2_ps[:], mybir.ActivationFunctionType.Sqrt,
                             scale=0.01)  # sqrt(0.01*nx2) = 0.1*||proj||
        inv10 = sb.tile([128, NT], F32)
        nc.vector.reciprocal(inv10[:], s01[:])

        # ---- matmul 2 + epilogue per frame-tile ----
        for t in range(NT):
            d_ps = ps.tile([128, N_CLS], F32, tag="dots_ps", bufs=2)
            for j in range(JT):
                nc.tensor.matmul(d_ps[:], projT[j][:, t * 128:(t + 1) * 128],
                                 embn_sb[j][:], start=(j == 0),
                                 stop=(j == JT - 1))
            o = sb.tile([128, 1 + N_CLS], F32, tag=f"o{t}")
            # cols 1..504: cos / 0.1  (row scale by 10/||proj[n]||)
            nc.scalar.activation(o[:, 1:], d_ps[:],
                                 mybir.ActivationFunctionType.Copy,
                                 scale=inv10[:, t:t + 1])
            # col 0: cos at the label column  (sum of cos * onehot);
            # the mul runs on the otherwise-idle GpSimd engine
            scr = sb.tile([128, N_CLS], F32, tag=f"scr{t}")
            nc.gpsimd.tensor_mul(scr[:], o[:, 1:], masku[t][:])
            nc.vector.reduce_sum(o[:, 0:1], scr[:], axis=mybir.AxisListType.X)
            # -inf at the label column among the negatives
            nc.vector.copy_predicated(o[:, 1:], masku[t][:], neginf[:])
            nc.sync.dma_start(out[t * 128:(t + 1) * 128, :], o[:])

    if split_waits:  # CoreSim can't model the injected NOPs; HW needs them
        _split_multi_waits(nc)
    return nc


def _prep_inputs(x, label, W, b, label_embeddings):
    x = np.asarray(x, dtype=np.float32)
    label = np.asarray(label)
    W = np.asarray(W, dtype=np.float32)
    b = np.asarray(b, dtype=np.float32)
    emb = np.asarray(label_embeddings, dtype=np.float32)

    WT = W.T                                             # [768, 256]
    embc = np.ascontiguousarray(
        emb.T.reshape(JT, 128, N_CLS).transpose(1, 0, 2).reshape(128, JT * N_CLS))
    iota = np.broadcast_to(np.arange(N_CLS, dtype=np.float32), (128, N_CLS))
    b2 = b.reshape(JT, 128).T                            # [128, 2]
    ones = np.ones((128, 128), np.float32)

    in_maps = []
    for i in range(N_CORES):
        lab_cols = label[i].astype(np.float32).reshape(NT, 128).T  # [128, 4]
        misc = np.ascontiguousarray(
            np.concatenate([lab_cols, iota, b2, ones], axis=1))  # [128, 638]
        wxi = np.ascontiguousarray(np.concatenate([WT, x[i].T], axis=1))
        in_maps.append({"wx": wxi, "embc": embc, "misc": misc})
    return in_maps


def kernel(x, label, mask_m, mask_u, W, b, label_embeddings, _trace=False):
    global _CACHED_NC
    if _CACHED_NC is None:
        _CACHED_NC = build_nc()
    nc = _CACHED_NC

    in_maps = _prep_inputs(x, label, W, b, label_embeddings)
    res = run_bass_kernel_spmd(nc, in_maps, list(range(N_CORES)), trace=_trace)

    full = np.concatenate([res.results[i]["out"][None] for i in range(N_CORES)])
    flat = full.reshape(B * T, 1 + N_CLS)
    m = np.asarray(mask_m).reshape(-1)
    u = np.asarray(mask_u).reshape(-1)
    outs = (flat[m], flat[u])
    if _trace:
        return outs, res
    return outs
